# revision 73
# baseline (speedup 1.0000x reference)
"""Clustered Linformer Attention — Trainium2 Bass kernel, 8 NeuronCores.

Strategy: data-parallel over batch (2 batches/core, no collectives).
Math restructuring (validated vs reference in errsim.py / mb3.py):
  - mask is all-ones => cluster c holds positions [32c, 32c+32), so the
    gather+einsum projections  k_proj_h = AE_h^T X wk_h  are computed as:
      (1) a "sparse sweep": for each 128-position chunk g, one matmul with
          a tiny host-built stationary table fwbd[:, g, :] in [128, 64]
          holding the E and F cluster weights block-diagonally
          (cols = 32*ef + 8*c_sub + h) -> kpEF[(g rows), D],
      (2) PE transposes of [128,128] blocks with a PERMUTATION matrix that
          simultaneously reorders rows into the block-diagonal
          [h0 clusters | h1 clusters] layout -> kpET/kpFT in [D, (hp, c)],
      (3) a small head-mix with wk/wv (contraction over D).
    This replaces dense [S,D]x[S,P*H] sweeps: 75% less PE work, and the
    4.2MB AE/AF tables shrink to a 0.26MB fwbd + 32KB perm.
  - the 3-kernel conv fusion over scores collapses to 5 "tap" matrices
    M_t in [P, P]:  scores_conv[s] = sum_t q[s+t] @ (k_proj^T M_t).
  - adjacent heads are packed block-diagonally so every matmul contracts
    over the full 128 partitions.
  - f16 replaces bf16 (8x lower base error), buying error budget for
    fp8e4m3 DoubleRow matmuls (2x PE throughput) on the score-only path:
    the q projection (contraction pairs over D) and the tap matmuls
    (tap pairs as DoubleRow planes; pairs (-2,-1),(0,1),(2,zero)).  kt is
    pre-scaled by 16 to stay clear of fp8 subnormals; the exp activation
    applies scale 1/16.
  - softmax Z = sum_c exp via an all-ones block-diag matmul broadcasting
    Z to all partitions; normalization is reciprocal + multiply.
  - final dense computed transposed (features on partitions) so dense_b
    folds into the scalar-engine PSUM->SBUF copy as a per-partition bias;
    output is f16 [D, S] per batch, transposed/cast to f32 on host.
Schedule: phase A = both batches' q/sweep/transpose/mix/kt (PE-dense),
phase B = both batches' attention (software-pipelined over the exp) with
the dense layers trailing one s-chunk behind.
"""
import sys
import numpy as np
import ml_dtypes

sys.path.insert(0, '/opt/trn_rl_repo')

B, S, D = 16, 2048, 512
H, P, C = 8, 64, 32
DEPTH = D // H           # 64
NCORES = 8
BLOC = B // NCORES       # 2 batches per core
NPAIR = H // 2           # 4 head pairs
SCH = 4                  # s-chunks of 512
SCW = S // SCH           # 512
NJ = S // 128            # 16 s-tiles of 128
NDC = D // 128           # 4 contraction chunks
SQ = S + 6               # padded q columns (2 front, 4 back)
KT_SCALE = 16.0

_CACHE = {}


def _build_nc():
    import concourse.tile as tile
    from concourse import mybir, bacc
    from concourse.ap import AP

    f32 = mybir.dt.float32
    f16 = mybir.dt.float16
    f8 = mybir.dt.float8e4
    DR = mybir.MatmulPerfMode.DoubleRow

    nc = bacc.Bacc()
    xT8 = nc.declare_dram_parameter("xT8", [BLOC, SCH, 128, NDC * SCW], f8,
                                    isOutput=False)
    x16 = nc.declare_dram_parameter("x16", [BLOC, 128, NJ * D], f16, isOutput=False)
    wq8 = nc.declare_dram_parameter("wq8", [128, NDC * D], f8, isOutput=False)
    wk = nc.declare_dram_parameter("wk", [128, NDC * D], f16, isOutput=False)
    wv = nc.declare_dram_parameter("wv", [128, NDC * D], f16, isOutput=False)
    dw = nc.declare_dram_parameter("dw", [128, NDC * D], f16, isOutput=False)
    dbb = nc.declare_dram_parameter("dbb", [128, NDC], f32, isOutput=False)
    fwbd = nc.declare_dram_parameter("fwbd", [128, NJ * 64], f16, isOutput=False)
    perm = nc.declare_dram_parameter("perm", [128, 128], f16, isOutput=False)
    bdm = nc.declare_dram_parameter("bdm", [128, 5 * 128], f16, isOutput=False)
    onesbd = nc.declare_dram_parameter("onesbd", [128, 128], f16, isOutput=False)
    out = nc.declare_dram_parameter("out", [BLOC, D, S], f16, isOutput=True)

    with tile.TileContext(nc) as tc:
        with tc.tile_pool(name="const", bufs=1) as cpool, \
             tc.tile_pool(name="big", bufs=1) as bigp, \
             tc.tile_pool(name="sm", bufs=4) as smp, \
             tc.tile_pool(name="ob", bufs=3) as obp, \
             tc.tile_pool(name="psB", bufs=6, space="PSUM") as psB, \
             tc.tile_pool(name="psS", bufs=2, space="PSUM") as psS:

            wq_sb = cpool.tile([128, NDC, D], f8)
            fw_sb = cpool.tile([128, NJ, 64], f16)
            pm_sb = cpool.tile([128, 128], f16)
            wk_sb = cpool.tile([128, NDC, D], f16)
            wv_sb = cpool.tile([128, NDC, D], f16)
            dw_sb = cpool.tile([128, NDC, D], f16)
            bdm_sb = cpool.tile([128, 5, 128], f16)
            ones_sb = cpool.tile([128, 128], f16)
            dbf = cpool.tile([128, NDC], f32)

            st = [dict() for _ in range(BLOC)]
            for b in range(BLOC):
                st[b]["xt8"] = bigp.tile([128, NDC, S], f8, tag="xt8", bufs=2,
                                         name=f"xt8_{b}")
                st[b]["x16"] = bigp.tile([128, NJ, D], f16, tag="x16", bufs=2,
                                         name=f"x16_{b}")
            xt8_0, x16_0 = st[0]["xt8"], st[0]["x16"]
            xt8_1, x16_1 = st[1]["xt8"], st[1]["x16"]
            # xT8 is n-major in DRAM: piece (b, n) = [128, NDC, 512] cols
            # 512n..512(n+1) of every dc chunk, so q chunks gate per piece.
            xt8v = [[xT8[b, n].rearrange("p (o c) -> p o c", o=NDC)
                     for n in range(SCH)] for b in range(BLOC)]

            def xt8_piece(b, n, eng):
                eng.dma_start(
                    out=st[b]["xt8"][:, :, SCW * n:SCW * (n + 1)],
                    in_=xt8v[b][n])
            x16v = [x16[b].rearrange("p (j d) -> p j d", j=NJ)
                    for b in range(BLOC)]
            JH = NJ // 2
            # --- DMA staging: scalar ring kept to 3 (engine has compute
            # duties and blocks if its ring fills); sync/gpsimd carry the
            # rest in deadline order.
            nc.scalar.dma_start(out=wq_sb, in_=wq8[:].rearrange("p (o m) -> p o m", o=NDC))
            xt8_piece(0, 1, nc.scalar)
            nc.scalar.dma_start(out=fw_sb, in_=fwbd[:].rearrange("p (j c) -> p j c", j=NJ))
            nc.scalar.dma_start(out=pm_sb, in_=perm[:])
            nc.scalar.dma_start(out=x16_0[:, JH:NJ, :], in_=x16v[0][:, JH:NJ, :])
            JQ = NJ // 4
            xt8_piece(0, 0, nc.sync)
            nc.sync.dma_start(out=x16_0[:, 0:JQ, :], in_=x16v[0][:, 0:JQ, :])
            xt8_piece(0, 3, nc.sync)
            nc.sync.dma_start(out=x16_0[:, JQ:JH, :], in_=x16v[0][:, JQ:JH, :])
            nc.sync.dma_start(out=wk_sb, in_=wk[:].rearrange("p (o m) -> p o m", o=NDC))
            nc.sync.dma_start(out=bdm_sb, in_=bdm[:].rearrange("p (t m) -> p t m", t=5))
            nc.sync.dma_start(out=ones_sb, in_=onesbd[:])
            nc.sync.dma_start(out=dbf, in_=dbb[:])
            nc.sync.dma_start(out=x16_1[:, 0:JH, :], in_=x16v[1][:, 0:JH, :])
            xt8_piece(0, 2, nc.gpsimd)
            nc.gpsimd.dma_start(out=wv_sb, in_=wv[:].rearrange("p (o m) -> p o m", o=NDC))
            for n in range(SCH):
                xt8_piece(1, n, nc.gpsimd)
            nc.gpsimd.dma_start(out=x16_1[:, JH:NJ, :], in_=x16v[1][:, JH:NJ, :])
            nc.gpsimd.dma_start(out=dw_sb, in_=dw[:].rearrange("p (o m) -> p o m", o=NDC))

            def emit_qt(b, pr, n):
                # qT chunk via fp8 DoubleRow: contraction pairs over dc.
                s = st[b]
                if pr == 0 and n == 0:
                    s["q8"] = bigp.tile([128, NPAIR, SQ], f8, tag="q8",
                                        bufs=2, name=f"q8_{b}")
                    nc.vector.memset(s["q8"][:, :, 0:2], 0.0)
                    nc.vector.memset(s["q8"][:, :, 2 + S:], 0.0)
                ps_q = psB.tile([128, SCW], f32, tag="ps512")
                for i in range(2):
                    nc.tensor.matmul(
                        ps_q,
                        wq_sb[:, 2 * i:2 * i + 2, 128 * pr:128 * (pr + 1)],
                        s["xt8"][:, 2 * i:2 * i + 2, SCW * n:SCW * (n + 1)],
                        start=(i == 0), stop=(i == 1), perf_mode=DR)
                nc.scalar.copy(
                    out=s["q8"][:, pr, 2 + SCW * n:2 + SCW * (n + 1)],
                    in_=ps_q)

            def emit_sweep(b, gp):
                # groups 2gp, 2gp+1: kpEF_sb[:, gp, :] rows 0-63 / 64-127
                s = st[b]
                if gp == 0:
                    s["kpEF"] = bigp.tile([128, JH, D], f16, tag="kpEF",
                                          bufs=2, name=f"kpEF_{b}")
                ps = psB.tile([128, D], f32, tag="ps512")
                for gh in range(2):
                    g = 2 * gp + gh
                    nc.tensor.matmul(ps[64 * gh:64 * (gh + 1), :],
                                     fw_sb[:, g, :], s["x16"][:, g, :],
                                     start=True, stop=True)
                nc.scalar.copy(out=s["kpEF"][:, gp, :], in_=ps)

            def emit_tpose(b, p, dc):
                # permuted transpose of kpEF block (pair p, D-slab dc) ->
                # kpT cols [ef-512, hp-slab-128, h'-64, 8 cols at 8p]
                s = st[b]
                if p == 0 and dc == 0:
                    s["kpT"] = bigp.tile([128, NDC, 2 * D], f16, tag="kpT",
                                         bufs=2, name=f"kpT_{b}")
                pst = psS.tile([128, 128], f16, tag="pssmall")
                nc.tensor.transpose(
                    pst, s["kpEF"][:, p, 128 * dc:128 * (dc + 1)], pm_sb)
                base = s["kpT"][:, dc, 0:2 * D]
                src = AP(tensor=pst.tensor, offset=pst.offset,
                         ap=[list(pst.ap[0]), [64, 2], [16, 4], [8, 2], [1, 8]])
                dst = AP(tensor=base.tensor, offset=base.offset + 8 * p,
                         ap=[list(base.ap[0]), [512, 2], [128, 4], [64, 2], [1, 8]])
                nc.vector.tensor_copy(out=dst, in_=src)

            def emit_mix(b, pr):
                # kp/vp diag blocks: kp = kpET_pr^T-contracted against wk_pr
                s = st[b]
                if pr == 0:
                    s["kp"] = bigp.tile([128, NPAIR, 128], f16, tag="kpbd",
                                        bufs=2, name=f"kp_{b}")
                    s["vp"] = bigp.tile([128, NPAIR, 128], f16, tag="vpbd",
                                        bufs=2, name=f"vp_{b}")
                    nc.vector.memset(s["kp"], 0.0)
                    nc.vector.memset(s["vp"], 0.0)
                cols = slice(128 * pr, 128 * (pr + 1))
                colsF = slice(D + 128 * pr, D + 128 * (pr + 1))
                for ecols, w_sb, dstk in ((cols, wk_sb, "kp"),
                                          (colsF, wv_sb, "vp")):
                    ps_p = psS.tile([128, 128], f32, tag="pssmall")
                    for dc in range(NDC):
                        nc.tensor.matmul(
                            ps_p, s["kpT"][:, dc, ecols], w_sb[:, dc, cols],
                            start=(dc == 0), stop=(dc == NDC - 1))
                    dst = s[dstk]
                    nc.vector.tensor_copy(
                        out=dst[0:64, pr, 0:64], in_=ps_p[0:64, 0:64])
                    nc.vector.tensor_copy(
                        out=dst[64:128, pr, 64:128], in_=ps_p[64:128, 64:128])

            def emit_kt(b, pr):
                # kt8[:, pr, pi, pl, :] = fp8(KT_SCALE * kt_t); taps laid
                # out as DoubleRow pairs (-2,-1),(0,1),(2,zero).
                s = st[b]
                if pr == 0:
                    s["kt8"] = bigp.tile([128, NPAIR, 3, 2, 128], f8,
                                         tag="kt8", bufs=2, name=f"kt8_{b}")
                    nc.vector.memset(s["kt8"][:, :, 2, 1, :], 0.0)
                    s["concat"] = bigp.tile([128, NPAIR, S], f16,
                                            tag="concatT", bufs=2,
                                            name=f"concat_{b}")
                for t in range(5):
                    ps_b = psS.tile([128, 128], f32, tag="pssmall")
                    nc.tensor.matmul(ps_b, s["kp"][:, pr, :], bdm_sb[:, t, :],
                                     start=True, stop=True)
                    nc.vector.tensor_scalar_mul(
                        s["kt8"][:, pr, t // 2, t % 2, :], ps_b, KT_SCALE)

            def _q_pair_ap(s, pr, n, pi):
                # rhs [128, 2, 512]: plane 0 = tap 2*pi-2, plane 1 = +1,
                # as an overlapping strided AP (plane stride = 1 column).
                base = s["q8"][:, pr, SCW * n + 2 * pi: SCW * n + 2 * pi + 512]
                ap = [list(p) for p in base.ap]
                assert len(ap) == 2 and ap[1][0] == 1
                return AP(tensor=base.tensor, offset=base.offset,
                          ap=[ap[0], [1, 2], [1, 512]])

            def emit_att_score(b, pr, n):
                # taps (3 fp8 DoubleRow matmuls) + exp on the scalar engine;
                # Z/AV deferred to emit_att_post (software pipelining).
                s = st[b]
                ps_sc = psB.tile([128, SCW], f32, tag="ps512")
                for pi in range(3):
                    nc.tensor.matmul(
                        ps_sc,
                        s["kt8"][:, pr, pi, :, :],
                        _q_pair_ap(s, pr, n, pi),
                        start=(pi == 0), stop=(pi == 2), perf_mode=DR)
                expt = smp.tile([128, SCW], f16, tag="expt")
                nc.scalar.activation(
                    out=expt, in_=ps_sc,
                    func=mybir.ActivationFunctionType.Exp,
                    scale=1.0 / KT_SCALE)
                return expt

            def emit_att_post(b, pr, n, expt):
                # Z/AV matmuls; normalize split across engines: reciprocal
                # on DVE, AV staged to SBUF by the scalar engine (gpsimd
                # cannot read PSUM), multiply on the otherwise-idle gpsimd.
                s = st[b]
                ps_z = psB.tile([128, SCW], f32, tag="ps512")
                nc.tensor.matmul(ps_z, ones_sb, expt, start=True, stop=True)
                ps_at = psB.tile([128, SCW], f32, tag="ps512")
                nc.tensor.matmul(ps_at, s["vp"][:, pr, :], expt,
                                 start=True, stop=True)
                rzb = smp.tile([128, SCW], f32, tag="rzb")
                nc.vector.reciprocal_approx_fast(out=rzb, in_=ps_z)
                atf = smp.tile([128, SCW], f16, tag="atf")
                nc.scalar.copy(out=atf, in_=ps_at)
                nc.gpsimd.tensor_mul(
                    out=s["concat"][:, pr, SCW * n:SCW * (n + 1)],
                    in0=atf, in1=rzb)

            def emit_dense(b, dblk, n):
                s = st[b]
                ps_d = psB.tile([128, SCW], f32, tag="ps512")
                cols = slice(128 * dblk, 128 * (dblk + 1))
                for dc in range(NDC):
                    nc.tensor.matmul(
                        ps_d,
                        dw_sb[:, dc, cols],
                        s["concat"][:, dc, SCW * n:SCW * (n + 1)],
                        start=(dc == 0), stop=(dc == NDC - 1))
                obuf = obp.tile([128, SCW], f16, tag="obuf")
                nc.scalar.activation(
                    out=obuf, in_=ps_d,
                    func=mybir.ActivationFunctionType.Identity,
                    bias=dbf[:, dblk:dblk + 1])
                eng = nc.sync if (dblk + n) % 2 == 0 else nc.gpsimd
                eng.dma_start(
                    out=out[b, cols, SCW * n:SCW * (n + 1)], in_=obuf)

            # ---- emission schedule ----
            # phase A0: batch-0 projections (q n-outer to match the n-major
            # xt8 piece arrivals); batch-1 q interleaved into the transposes
            # to hide their PSUM-reuse stalls.
            for n in range(SCH):
                for pr in range(NPAIR):
                    emit_qt(0, pr, n)
            for gp in range(JH):
                emit_sweep(0, gp)
            q1 = [(pr, n) for n in range(SCH) for pr in range(NPAIR)]
            q1i = 0
            for p in range(JH):
                for dc in range(NDC):
                    emit_tpose(0, p, dc)
                    if q1i < len(q1) and (p * NDC + dc) % 2 == 0:
                        emit_qt(1, *q1[q1i]); q1i += 1
            for pr in range(NPAIR):
                emit_mix(0, pr)
            while q1i < len(q1):
                emit_qt(1, *q1[q1i]); q1i += 1
            for pr in range(NPAIR):
                emit_kt(0, pr)
            # phase A1: batch-1 sweeps (transposes deferred into B0)
            for gp in range(JH):
                emit_sweep(1, gp)
            # phase B0: batch-0 attention (depth-2 software pipeline over
            # the exp) with batch-1 transposes as PE fillers
            tp1 = [(p, dc) for p in range(JH) for dc in range(NDC)]
            ti = 0
            prevs = []
            for n in range(SCH):
                for pr in range(NPAIR):
                    expt = emit_att_score(0, pr, n)
                    for _ in range(2):
                        if ti < len(tp1):
                            emit_tpose(1, *tp1[ti]); ti += 1
                    if len(prevs) == 2:
                        emit_att_post(0, *prevs.pop(0))
                    prevs.append((pr, n, expt))
            for pv in prevs:
                emit_att_post(0, *pv)
            dense_q = [(0, dblk, n) for n in range(SCH) for dblk in range(NDC)]
            while ti < len(tp1):
                emit_tpose(1, *tp1[ti]); ti += 1
            for pr in range(NPAIR):
                emit_mix(1, pr)
            for pr in range(NPAIR):
                emit_kt(1, pr)
            # phase B1: batch-1 attention with remaining batch-0 dense as
            # fillers; batch-1 dense trails one s-chunk behind.
            prev = None
            for n in range(SCH):
                for pr in range(NPAIR):
                    expt = emit_att_score(1, pr, n)
                    if dense_q:
                        emit_dense(*dense_q.pop(0))
                    if prev is not None:
                        emit_att_post(1, *prev)
                        if prev[0] == NPAIR - 1:
                            dense_q.extend((1, dblk, prev[1])
                                           for dblk in range(NDC))
                    prev = (pr, n, expt)
            emit_att_post(1, *prev)
            dense_q.extend((1, dblk, SCH - 1) for dblk in range(NDC))
            while dense_q:
                emit_dense(*dense_q.pop(0))

    nc.finalize()
    return nc


def _prep_inputs(x, mask, wq, wk, wv, EW, FW, conv_w1, conv_w3, conv_w5, conv_b,
                 dense_w, dense_b, cluster_table):
    """Host-side restructuring -> per-core input maps."""
    f8 = ml_dtypes.float8_e4m3
    x = np.ascontiguousarray(np.asarray(x, np.float32))
    mask = np.asarray(mask)
    counts = np.clip(mask.astype(np.int64).sum(1), 1, S)
    pos = np.asarray(cluster_table)[counts - 1]          # [B, P, C]
    if not (pos == pos[0]).all():
        raise NotImplementedError("per-batch cluster tables not supported")
    p0 = pos[0]                                          # [P, C]
    if not (p0 == (np.arange(P)[:, None] * C + np.arange(C)[None, :])).all():
        raise NotImplementedError("non-contiguous clusters not supported")

    scale = 1.0 / np.sqrt(np.float32(DEPTH))
    EWs = np.asarray(EW, np.float32) * scale             # [H, P, C]
    FWs = np.asarray(FW, np.float32)

    # fwbd[pos_in_chunk, g, 32*ef + 8*c_sub + h] = table[h, 4g+c_sub, l]
    # where pos_in_chunk = 32*c_sub + l
    FWBD = np.zeros((128, NJ, 64), np.float32)
    l = np.arange(C)
    for g in range(NJ):
        for c_sub in range(4):
            c = 4 * g + c_sub
            for ef, tab in ((0, EWs), (1, FWs)):
                FWBD[32 * c_sub + l[:, None], g,
                     32 * ef + 8 * c_sub + np.arange(H)[None, :]] = \
                    tab[:, c, :].T
    # permutation: row (gpar, ef, c_sub, h) -> col
    # 64*ef + 16*(h>>1) + 8*(h&1) + 4*gpar + c_sub
    PM = np.zeros((128, 128), np.float32)
    for gpar in range(2):
        for ef in range(2):
            for c_sub in range(4):
                for h in range(H):
                    r = 64 * gpar + 32 * ef + 8 * c_sub + h
                    col = 64 * ef + 16 * (h >> 1) + 8 * (h & 1) + \
                        4 * gpar + c_sub
                    PM[r, col] = 1.0

    # conv -> 5 tap matrices
    wp = np.arange(P)[:, None]
    jj = np.arange(P)[None, :]
    ii = wp - jj + 31
    valid = (ii >= 0) & (ii < P)
    ii = np.clip(ii, 0, P - 1)
    M = {t: np.zeros((P, P), np.float32) for t in range(-2, 3)}
    for cw, hk in ((conv_w1, 1), (conv_w3, 3), (conv_w5, 5)):
        cw = np.asarray(cw, np.float32)
        pad = (hk - 1) // 2
        for dy in range(hk):
            filt = cw[dy, :, 0, 0]
            M[dy - pad] += np.where(valid, filt[ii], 0.0) / 3.0
    BDM = np.zeros((5, 128, 128), np.float32)
    for ti in range(5):
        BDM[ti, :64, :64] = M[ti - 2]
        BDM[ti, 64:, 64:] = M[ti - 2]
    bbar = float(np.asarray(conv_b, np.float32).mean())
    if abs(bbar) > 1e-30:
        raise NotImplementedError("nonzero conv bias not folded")

    ones_bd = np.zeros((128, 128), np.float32)
    ones_bd[:64, :64] = 1.0
    ones_bd[64:, 64:] = 1.0

    def pm_o(w):
        # [O*128, M] -> [128, O*M]  (partition = inner row index)
        w = np.asarray(w, np.float32)
        o = w.shape[0] // 128
        return np.ascontiguousarray(
            w.reshape(o, 128, -1).transpose(1, 0, 2).reshape(128, -1))

    xsh = x.reshape(NCORES, BLOC, S, D)
    in_maps = []
    shared = dict(
        wq8=pm_o(wq).astype(f8),
        wk=pm_o(wk).astype(np.float16),
        wv=pm_o(wv).astype(np.float16),
        dw=pm_o(dense_w).astype(np.float16),
        dbb=np.ascontiguousarray(np.asarray(dense_b, np.float32)
                                 .reshape(NDC, 128).T),
        fwbd=np.ascontiguousarray(FWBD.reshape(128, NJ * 64)).astype(np.float16),
        perm=PM.astype(np.float16),
        bdm=np.ascontiguousarray(
            BDM.transpose(1, 0, 2).reshape(128, 5 * 128)).astype(np.float16),
        onesbd=ones_bd.astype(np.float16),
    )
    for c in range(NCORES):
        m = dict(shared)
        xT = xsh[c].transpose(0, 2, 1)           # [BLOC, D, S]
        m["xT8"] = np.ascontiguousarray(
            xT.reshape(BLOC, NDC, 128, SCH, SCW).transpose(0, 3, 2, 1, 4)
            .reshape(BLOC, SCH, 128, NDC * SCW)).astype(f8)
        m["x16"] = np.ascontiguousarray(
            xsh[c].reshape(BLOC, NJ, 128, D).transpose(0, 2, 1, 3)
            .reshape(BLOC, 128, NJ * D)).astype(np.float16)
        in_maps.append(m)
    return in_maps


def _run(in_maps, trace=False, tmpdir=None):
    from concourse.bass_utils import run_bass_kernel_spmd
    if "nc" not in _CACHE:
        _CACHE["nc"] = _build_nc()
    kw = {}
    if trace:
        _install_ntff_hook()
        kw = dict(trace=True, tmpdir=tmpdir)
    return run_bass_kernel_spmd(_CACHE["nc"], in_maps,
                                core_ids=list(range(NCORES)), **kw)


def _install_ntff_hook():
    import types, importlib.util as ilu
    if "antenv.axon_hooks" in sys.modules:
        return
    spec = ilu.spec_from_file_location(
        "trn_boot_mod", "/root/.axon_site/trn_agent_boot/trn_boot.py")
    tb = ilu.module_from_spec(spec)
    spec.loader.exec_module(tb)
    hook = tb._ntff_profile_via_ctypes("/opt/axon/libaxon_pjrt.so")
    mod = types.ModuleType("antenv.axon_hooks")
    mod.get_axon_ntff_profile_hook = lambda: hook
    import antenv  # noqa: F401
    sys.modules["antenv.axon_hooks"] = mod


def kernel(**inputs) -> np.ndarray:
    in_maps = _prep_inputs(**inputs)
    r = _run(in_maps)
    outs = [np.ascontiguousarray(
        r.results[c]["out"].transpose(0, 2, 1)).astype(np.float32)
        for c in range(NCORES)]
    return np.concatenate(outs, axis=0)


# revision 76
# speedup vs baseline: 1.0278x; 1.0278x over previous
"""Clustered Linformer Attention — Trainium2 Bass kernel, 8 NeuronCores.

Strategy: data-parallel over batch (2 batches/core, no collectives).
Math restructuring (validated vs reference in errsim.py / mb3.py):
  - mask is all-ones => cluster c holds positions [32c, 32c+32), so the
    gather+einsum projections  k_proj_h = AE_h^T X wk_h  are computed as:
      (1) a "sparse sweep": for each 128-position chunk g, one matmul with
          a tiny host-built stationary table fwbd[:, g, :] in [128, 64]
          holding the E and F cluster weights block-diagonally
          (cols = 32*ef + 8*c_sub + h) -> kpEF[(g rows), D],
      (2) PE transposes of [128,128] blocks with a PERMUTATION matrix that
          simultaneously reorders rows into the block-diagonal
          [h0 clusters | h1 clusters] layout -> kpET/kpFT in [D, (hp, c)],
      (3) a small head-mix with wk/wv (contraction over D).
    This replaces dense [S,D]x[S,P*H] sweeps: 75% less PE work, and the
    4.2MB AE/AF tables shrink to a 0.26MB fwbd + 32KB perm.
  - the 3-kernel conv fusion over scores collapses to 5 "tap" matrices
    M_t in [P, P]:  scores_conv[s] = sum_t q[s+t] @ (k_proj^T M_t).
  - adjacent heads are packed block-diagonally so every matmul contracts
    over the full 128 partitions.
  - f16 replaces bf16 (8x lower base error), buying error budget for
    fp8e4m3 DoubleRow matmuls (2x PE throughput) on the score-only path:
    the q projection (contraction pairs over D) and the tap matmuls
    (tap pairs as DoubleRow planes; pairs (-2,-1),(0,1),(2,zero)).  kt is
    pre-scaled by 16 to stay clear of fp8 subnormals; the exp activation
    applies scale 1/16.
  - softmax Z = sum_c exp via an all-ones block-diag matmul broadcasting
    Z to all partitions; normalization is reciprocal + multiply.
  - final dense computed transposed (features on partitions) so dense_b
    folds into the scalar-engine PSUM->SBUF copy as a per-partition bias;
    output is f16 [D, S] per batch, transposed/cast to f32 on host.
Schedule: phase A = both batches' q/sweep/transpose/mix/kt (PE-dense),
phase B = both batches' attention (software-pipelined over the exp) with
the dense layers trailing one s-chunk behind.
"""
import sys
import numpy as np
import ml_dtypes

sys.path.insert(0, '/opt/trn_rl_repo')

B, S, D = 16, 2048, 512
H, P, C = 8, 64, 32
DEPTH = D // H           # 64
NCORES = 8
BLOC = B // NCORES       # 2 batches per core
NPAIR = H // 2           # 4 head pairs
SCH = 4                  # s-chunks of 512
SCW = S // SCH           # 512
NJ = S // 128            # 16 s-tiles of 128
NDC = D // 128           # 4 contraction chunks
SQ = S + 6               # padded q columns (2 front, 4 back)
KT_SCALE = 16.0

_CACHE = {}


def _build_nc():
    import concourse.tile as tile
    from concourse import mybir, bacc
    from concourse.ap import AP

    f32 = mybir.dt.float32
    f16 = mybir.dt.float16
    f8 = mybir.dt.float8e4
    DR = mybir.MatmulPerfMode.DoubleRow

    nc = bacc.Bacc()
    xT8 = nc.declare_dram_parameter("xT8", [BLOC, SCH, 128, NDC * SCW], f8,
                                    isOutput=False)
    x16 = nc.declare_dram_parameter("x16", [BLOC, 128, NJ * D], f16, isOutput=False)
    wq8 = nc.declare_dram_parameter("wq8", [128, NDC * D], f8, isOutput=False)
    wk = nc.declare_dram_parameter("wk", [128, NDC * D], f16, isOutput=False)
    wv = nc.declare_dram_parameter("wv", [128, NDC * D], f16, isOutput=False)
    dw = nc.declare_dram_parameter("dw", [128, NDC * D], f16, isOutput=False)
    dbb = nc.declare_dram_parameter("dbb", [128, NDC], f32, isOutput=False)
    fwbd = nc.declare_dram_parameter("fwbd", [128, NJ * 64], f16, isOutput=False)
    perm = nc.declare_dram_parameter("perm", [128, 128], f16, isOutput=False)
    bdm = nc.declare_dram_parameter("bdm", [128, 5 * 128], f16, isOutput=False)
    onesbd = nc.declare_dram_parameter("onesbd", [128, 128], f16, isOutput=False)
    out = nc.declare_dram_parameter("out", [BLOC, D, S], f16, isOutput=True)

    with tile.TileContext(nc) as tc:
        with tc.tile_pool(name="const", bufs=1) as cpool, \
             tc.tile_pool(name="big", bufs=1) as bigp, \
             tc.tile_pool(name="sm", bufs=4) as smp, \
             tc.tile_pool(name="ob", bufs=3) as obp, \
             tc.tile_pool(name="psB", bufs=6, space="PSUM") as psB, \
             tc.tile_pool(name="psS", bufs=2, space="PSUM") as psS:

            wq_sb = cpool.tile([128, NDC, D], f8)
            fw_sb = cpool.tile([128, NJ, 64], f16)
            pm_sb = cpool.tile([128, 128], f16)
            wk_sb = cpool.tile([128, NDC, D], f16)
            wv_sb = cpool.tile([128, NDC, D], f16)
            dw_sb = cpool.tile([128, NDC, D], f16)
            bdm_sb = cpool.tile([128, 5, 128], f16)
            ones_sb = cpool.tile([128, 128], f16)
            dbf = cpool.tile([128, NDC], f32)

            st = [dict() for _ in range(BLOC)]
            for b in range(BLOC):
                st[b]["xt8"] = bigp.tile([128, NDC, S], f8, tag="xt8", bufs=2,
                                         name=f"xt8_{b}")
                st[b]["x16"] = bigp.tile([128, NJ, D], f16, tag="x16", bufs=2,
                                         name=f"x16_{b}")
            xt8_0, x16_0 = st[0]["xt8"], st[0]["x16"]
            xt8_1, x16_1 = st[1]["xt8"], st[1]["x16"]
            # xT8 is n-major in DRAM: piece (b, n) = [128, NDC, 512] cols
            # 512n..512(n+1) of every dc chunk, so q chunks gate per piece.
            xt8v = [[xT8[b, n].rearrange("p (o c) -> p o c", o=NDC)
                     for n in range(SCH)] for b in range(BLOC)]

            def xt8_piece(b, n, eng):
                eng.dma_start(
                    out=st[b]["xt8"][:, :, SCW * n:SCW * (n + 1)],
                    in_=xt8v[b][n])
            x16v = [x16[b].rearrange("p (j d) -> p j d", j=NJ)
                    for b in range(BLOC)]
            JH = NJ // 2
            # --- DMA staging: scalar ring kept to 3 (engine has compute
            # duties and blocks if its ring fills); sync/gpsimd carry the
            # rest in deadline order.
            fwv = fwbd[:].rearrange("p (j c) -> p j c", j=NJ)
            nc.scalar.dma_start(out=wq_sb, in_=wq8[:].rearrange("p (o m) -> p o m", o=NDC))
            xt8_piece(0, 1, nc.scalar)
            nc.scalar.dma_start(out=fw_sb[:, JH:NJ, :], in_=fwv[:, JH:NJ, :])
            nc.scalar.dma_start(out=x16_0[:, JH:NJ, :], in_=x16v[0][:, JH:NJ, :])
            JQ = NJ // 4
            xt8_piece(0, 0, nc.sync)
            nc.sync.dma_start(out=x16_0[:, 0:JQ, :], in_=x16v[0][:, 0:JQ, :])
            nc.sync.dma_start(out=fw_sb[:, 0:JH, :], in_=fwv[:, 0:JH, :])
            xt8_piece(0, 3, nc.sync)
            nc.sync.dma_start(out=x16_0[:, JQ:JH, :], in_=x16v[0][:, JQ:JH, :])
            nc.sync.dma_start(out=wk_sb, in_=wk[:].rearrange("p (o m) -> p o m", o=NDC))
            nc.sync.dma_start(out=bdm_sb, in_=bdm[:].rearrange("p (t m) -> p t m", t=5))
            nc.sync.dma_start(out=ones_sb, in_=onesbd[:])
            nc.sync.dma_start(out=dbf, in_=dbb[:])
            nc.sync.dma_start(out=x16_1[:, 0:JH, :], in_=x16v[1][:, 0:JH, :])
            xt8_piece(0, 2, nc.gpsimd)
            nc.gpsimd.dma_start(out=pm_sb, in_=perm[:])
            nc.gpsimd.dma_start(out=wv_sb, in_=wv[:].rearrange("p (o m) -> p o m", o=NDC))
            for n in range(SCH):
                xt8_piece(1, n, nc.gpsimd)
            nc.gpsimd.dma_start(out=x16_1[:, JH:NJ, :], in_=x16v[1][:, JH:NJ, :])
            nc.gpsimd.dma_start(out=dw_sb, in_=dw[:].rearrange("p (o m) -> p o m", o=NDC))

            def emit_qt(b, pr, n):
                # qT chunk via fp8 DoubleRow: contraction pairs over dc.
                s = st[b]
                if pr == 0 and n == 0:
                    s["q8"] = bigp.tile([128, NPAIR, SQ], f8, tag="q8",
                                        bufs=2, name=f"q8_{b}")
                    nc.vector.memset(s["q8"][:, :, 0:2], 0.0)
                    nc.vector.memset(s["q8"][:, :, 2 + S:], 0.0)
                ps_q = psB.tile([128, SCW], f32, tag="ps512")
                for i in range(2):
                    nc.tensor.matmul(
                        ps_q,
                        wq_sb[:, 2 * i:2 * i + 2, 128 * pr:128 * (pr + 1)],
                        s["xt8"][:, 2 * i:2 * i + 2, SCW * n:SCW * (n + 1)],
                        start=(i == 0), stop=(i == 1), perf_mode=DR)
                nc.scalar.copy(
                    out=s["q8"][:, pr, 2 + SCW * n:2 + SCW * (n + 1)],
                    in_=ps_q)

            def emit_sweep(b, gp):
                # groups 2gp, 2gp+1: kpEF_sb[:, gp, :] rows 0-63 / 64-127
                s = st[b]
                if gp == 0:
                    s["kpEF"] = bigp.tile([128, JH, D], f16, tag="kpEF",
                                          bufs=2, name=f"kpEF_{b}")
                ps = psB.tile([128, D], f32, tag="ps512")
                for gh in range(2):
                    g = 2 * gp + gh
                    nc.tensor.matmul(ps[64 * gh:64 * (gh + 1), :],
                                     fw_sb[:, g, :], s["x16"][:, g, :],
                                     start=True, stop=True)
                nc.scalar.copy(out=s["kpEF"][:, gp, :], in_=ps)

            def emit_tpose(b, p, dc):
                # permuted transpose of kpEF block (pair p, D-slab dc) ->
                # kpT cols [ef-512, hp-slab-128, h'-64, 8 cols at 8p]
                s = st[b]
                if p == 0 and dc == 0:
                    s["kpT"] = bigp.tile([128, NDC, 2 * D], f16, tag="kpT",
                                         bufs=2, name=f"kpT_{b}")
                pst = psS.tile([128, 128], f16, tag="pssmall")
                nc.tensor.transpose(
                    pst, s["kpEF"][:, p, 128 * dc:128 * (dc + 1)], pm_sb)
                base = s["kpT"][:, dc, 0:2 * D]
                src = AP(tensor=pst.tensor, offset=pst.offset,
                         ap=[list(pst.ap[0]), [64, 2], [16, 4], [8, 2], [1, 8]])
                dst = AP(tensor=base.tensor, offset=base.offset + 8 * p,
                         ap=[list(base.ap[0]), [512, 2], [128, 4], [64, 2], [1, 8]])
                nc.vector.tensor_copy(out=dst, in_=src)

            def emit_mix(b, pr):
                # kp/vp diag blocks: kp = kpET_pr^T-contracted against wk_pr
                s = st[b]
                if pr == 0:
                    s["kp"] = bigp.tile([128, NPAIR, 128], f16, tag="kpbd",
                                        bufs=2, name=f"kp_{b}")
                    s["vp"] = bigp.tile([128, NPAIR, 128], f16, tag="vpbd",
                                        bufs=2, name=f"vp_{b}")
                    nc.vector.memset(s["kp"], 0.0)
                    nc.vector.memset(s["vp"], 0.0)
                cols = slice(128 * pr, 128 * (pr + 1))
                colsF = slice(D + 128 * pr, D + 128 * (pr + 1))
                for ecols, w_sb, dstk in ((cols, wk_sb, "kp"),
                                          (colsF, wv_sb, "vp")):
                    ps_p = psS.tile([128, 128], f32, tag="pssmall")
                    for dc in range(NDC):
                        nc.tensor.matmul(
                            ps_p, s["kpT"][:, dc, ecols], w_sb[:, dc, cols],
                            start=(dc == 0), stop=(dc == NDC - 1))
                    dst = s[dstk]
                    nc.vector.tensor_copy(
                        out=dst[0:64, pr, 0:64], in_=ps_p[0:64, 0:64])
                    nc.vector.tensor_copy(
                        out=dst[64:128, pr, 64:128], in_=ps_p[64:128, 64:128])

            def emit_kt(b, pr):
                # kt8[:, pr, pi, pl, :] = fp8(KT_SCALE * kt_t); taps laid
                # out as DoubleRow pairs (-2,-1),(0,1),(2,zero).
                s = st[b]
                if pr == 0:
                    s["kt8"] = bigp.tile([128, NPAIR, 3, 2, 128], f8,
                                         tag="kt8", bufs=2, name=f"kt8_{b}")
                    nc.vector.memset(s["kt8"][:, :, 2, 1, :], 0.0)
                    s["concat"] = bigp.tile([128, NPAIR, S], f16,
                                            tag="concatT", bufs=2,
                                            name=f"concat_{b}")
                for t in range(5):
                    ps_b = psS.tile([128, 128], f32, tag="pssmall")
                    nc.tensor.matmul(ps_b, s["kp"][:, pr, :], bdm_sb[:, t, :],
                                     start=True, stop=True)
                    nc.vector.tensor_scalar_mul(
                        s["kt8"][:, pr, t // 2, t % 2, :], ps_b, KT_SCALE)

            def _q_pair_ap(s, pr, n, pi):
                # rhs [128, 2, 512]: plane 0 = tap 2*pi-2, plane 1 = +1,
                # as an overlapping strided AP (plane stride = 1 column).
                base = s["q8"][:, pr, SCW * n + 2 * pi: SCW * n + 2 * pi + 512]
                ap = [list(p) for p in base.ap]
                assert len(ap) == 2 and ap[1][0] == 1
                return AP(tensor=base.tensor, offset=base.offset,
                          ap=[ap[0], [1, 2], [1, 512]])

            def emit_att_score(b, pr, n):
                # taps (3 fp8 DoubleRow matmuls) + exp on the scalar engine;
                # Z/AV deferred to emit_att_post (software pipelining).
                s = st[b]
                ps_sc = psB.tile([128, SCW], f32, tag="ps512")
                for pi in range(3):
                    nc.tensor.matmul(
                        ps_sc,
                        s["kt8"][:, pr, pi, :, :],
                        _q_pair_ap(s, pr, n, pi),
                        start=(pi == 0), stop=(pi == 2), perf_mode=DR)
                expt = smp.tile([128, SCW], f16, tag="expt")
                nc.scalar.activation(
                    out=expt, in_=ps_sc,
                    func=mybir.ActivationFunctionType.Exp,
                    scale=1.0 / KT_SCALE)
                return expt

            def emit_att_post(b, pr, n, expt):
                # Z/AV matmuls; normalize split across engines: reciprocal
                # on DVE, AV staged to SBUF by the scalar engine (gpsimd
                # cannot read PSUM), multiply on the otherwise-idle gpsimd.
                s = st[b]
                ps_z = psB.tile([128, SCW], f32, tag="ps512")
                nc.tensor.matmul(ps_z, ones_sb, expt, start=True, stop=True)
                ps_at = psB.tile([128, SCW], f32, tag="ps512")
                nc.tensor.matmul(ps_at, s["vp"][:, pr, :], expt,
                                 start=True, stop=True)
                rzb = smp.tile([128, SCW], f32, tag="rzb")
                nc.vector.reciprocal_approx_fast(out=rzb, in_=ps_z)
                atf = smp.tile([128, SCW], f16, tag="atf")
                nc.scalar.copy(out=atf, in_=ps_at)
                nc.gpsimd.tensor_mul(
                    out=s["concat"][:, pr, SCW * n:SCW * (n + 1)],
                    in0=atf, in1=rzb)

            def emit_dense(b, dblk, n):
                s = st[b]
                ps_d = psB.tile([128, SCW], f32, tag="ps512")
                cols = slice(128 * dblk, 128 * (dblk + 1))
                for dc in range(NDC):
                    nc.tensor.matmul(
                        ps_d,
                        dw_sb[:, dc, cols],
                        s["concat"][:, dc, SCW * n:SCW * (n + 1)],
                        start=(dc == 0), stop=(dc == NDC - 1))
                obuf = obp.tile([128, SCW], f16, tag="obuf")
                nc.scalar.activation(
                    out=obuf, in_=ps_d,
                    func=mybir.ActivationFunctionType.Identity,
                    bias=dbf[:, dblk:dblk + 1])
                eng = nc.sync if (dblk + n) % 2 == 0 else nc.gpsimd
                eng.dma_start(
                    out=out[b, cols, SCW * n:SCW * (n + 1)], in_=obuf)

            # ---- emission schedule ----
            # phase A0: batch-0 projections (q n-outer to match the n-major
            # xt8 piece arrivals); batch-1 q interleaved into the transposes
            # to hide their PSUM-reuse stalls.
            for n in range(SCH):
                for pr in range(NPAIR):
                    emit_qt(0, pr, n)
            for gp in range(JH):
                emit_sweep(0, gp)
            q1 = [(pr, n) for n in range(SCH) for pr in range(NPAIR)]
            q1i = 0
            for p in range(JH):
                for dc in range(NDC):
                    emit_tpose(0, p, dc)
                    if q1i < len(q1) and (p * NDC + dc) % 2 == 0:
                        emit_qt(1, *q1[q1i]); q1i += 1
            for pr in range(NPAIR):
                emit_mix(0, pr)
            while q1i < len(q1):
                emit_qt(1, *q1[q1i]); q1i += 1
            for pr in range(NPAIR):
                emit_kt(0, pr)
            # phase A1: batch-1 sweeps (transposes deferred into B0)
            for gp in range(JH):
                emit_sweep(1, gp)
            # phase B0: batch-0 attention (depth-2 software pipeline over
            # the exp) with batch-1 transposes as PE fillers
            tp1 = [(p, dc) for p in range(JH) for dc in range(NDC)]
            ti = 0
            prevs = []
            for n in range(SCH):
                for pr in range(NPAIR):
                    expt = emit_att_score(0, pr, n)
                    for _ in range(2):
                        if ti < len(tp1):
                            emit_tpose(1, *tp1[ti]); ti += 1
                    if len(prevs) == 2:
                        emit_att_post(0, *prevs.pop(0))
                    prevs.append((pr, n, expt))
            for pv in prevs:
                emit_att_post(0, *pv)
            dense_q = [(0, dblk, n) for n in range(SCH) for dblk in range(NDC)]
            while ti < len(tp1):
                emit_tpose(1, *tp1[ti]); ti += 1
            for pr in range(NPAIR):
                emit_mix(1, pr)
            for pr in range(NPAIR):
                emit_kt(1, pr)
            # phase B1: batch-1 attention with remaining batch-0 dense as
            # fillers; batch-1 dense trails one s-chunk behind.
            prev = None
            for n in range(SCH):
                for pr in range(NPAIR):
                    expt = emit_att_score(1, pr, n)
                    if dense_q:
                        emit_dense(*dense_q.pop(0))
                    if prev is not None:
                        emit_att_post(1, *prev)
                        if prev[0] == NPAIR - 1:
                            dense_q.extend((1, dblk, prev[1])
                                           for dblk in range(NDC))
                    prev = (pr, n, expt)
            emit_att_post(1, *prev)
            dense_q.extend((1, dblk, SCH - 1) for dblk in range(NDC))
            while dense_q:
                emit_dense(*dense_q.pop(0))

    nc.finalize()
    return nc


def _prep_inputs(x, mask, wq, wk, wv, EW, FW, conv_w1, conv_w3, conv_w5, conv_b,
                 dense_w, dense_b, cluster_table):
    """Host-side restructuring -> per-core input maps."""
    f8 = ml_dtypes.float8_e4m3
    x = np.ascontiguousarray(np.asarray(x, np.float32))
    mask = np.asarray(mask)
    counts = np.clip(mask.astype(np.int64).sum(1), 1, S)
    pos = np.asarray(cluster_table)[counts - 1]          # [B, P, C]
    if not (pos == pos[0]).all():
        raise NotImplementedError("per-batch cluster tables not supported")
    p0 = pos[0]                                          # [P, C]
    if not (p0 == (np.arange(P)[:, None] * C + np.arange(C)[None, :])).all():
        raise NotImplementedError("non-contiguous clusters not supported")

    scale = 1.0 / np.sqrt(np.float32(DEPTH))
    EWs = np.asarray(EW, np.float32) * scale             # [H, P, C]
    FWs = np.asarray(FW, np.float32)

    # fwbd[pos_in_chunk, g, 32*ef + 8*c_sub + h] = table[h, 4g+c_sub, l]
    # where pos_in_chunk = 32*c_sub + l
    FWBD = np.zeros((128, NJ, 64), np.float32)
    l = np.arange(C)
    for g in range(NJ):
        for c_sub in range(4):
            c = 4 * g + c_sub
            for ef, tab in ((0, EWs), (1, FWs)):
                FWBD[32 * c_sub + l[:, None], g,
                     32 * ef + 8 * c_sub + np.arange(H)[None, :]] = \
                    tab[:, c, :].T
    # permutation: row (gpar, ef, c_sub, h) -> col
    # 64*ef + 16*(h>>1) + 8*(h&1) + 4*gpar + c_sub
    PM = np.zeros((128, 128), np.float32)
    for gpar in range(2):
        for ef in range(2):
            for c_sub in range(4):
                for h in range(H):
                    r = 64 * gpar + 32 * ef + 8 * c_sub + h
                    col = 64 * ef + 16 * (h >> 1) + 8 * (h & 1) + \
                        4 * gpar + c_sub
                    PM[r, col] = 1.0

    # conv -> 5 tap matrices
    wp = np.arange(P)[:, None]
    jj = np.arange(P)[None, :]
    ii = wp - jj + 31
    valid = (ii >= 0) & (ii < P)
    ii = np.clip(ii, 0, P - 1)
    M = {t: np.zeros((P, P), np.float32) for t in range(-2, 3)}
    for cw, hk in ((conv_w1, 1), (conv_w3, 3), (conv_w5, 5)):
        cw = np.asarray(cw, np.float32)
        pad = (hk - 1) // 2
        for dy in range(hk):
            filt = cw[dy, :, 0, 0]
            M[dy - pad] += np.where(valid, filt[ii], 0.0) / 3.0
    BDM = np.zeros((5, 128, 128), np.float32)
    for ti in range(5):
        BDM[ti, :64, :64] = M[ti - 2]
        BDM[ti, 64:, 64:] = M[ti - 2]
    bbar = float(np.asarray(conv_b, np.float32).mean())
    if abs(bbar) > 1e-30:
        raise NotImplementedError("nonzero conv bias not folded")

    ones_bd = np.zeros((128, 128), np.float32)
    ones_bd[:64, :64] = 1.0
    ones_bd[64:, 64:] = 1.0

    def pm_o(w):
        # [O*128, M] -> [128, O*M]  (partition = inner row index)
        w = np.asarray(w, np.float32)
        o = w.shape[0] // 128
        return np.ascontiguousarray(
            w.reshape(o, 128, -1).transpose(1, 0, 2).reshape(128, -1))

    xsh = x.reshape(NCORES, BLOC, S, D)
    in_maps = []
    shared = dict(
        wq8=pm_o(wq).astype(f8),
        wk=pm_o(wk).astype(np.float16),
        wv=pm_o(wv).astype(np.float16),
        dw=pm_o(dense_w).astype(np.float16),
        dbb=np.ascontiguousarray(np.asarray(dense_b, np.float32)
                                 .reshape(NDC, 128).T),
        fwbd=np.ascontiguousarray(FWBD.reshape(128, NJ * 64)).astype(np.float16),
        perm=PM.astype(np.float16),
        bdm=np.ascontiguousarray(
            BDM.transpose(1, 0, 2).reshape(128, 5 * 128)).astype(np.float16),
        onesbd=ones_bd.astype(np.float16),
    )
    for c in range(NCORES):
        m = dict(shared)
        xT = xsh[c].transpose(0, 2, 1)           # [BLOC, D, S]
        m["xT8"] = np.ascontiguousarray(
            xT.reshape(BLOC, NDC, 128, SCH, SCW).transpose(0, 3, 2, 1, 4)
            .reshape(BLOC, SCH, 128, NDC * SCW)).astype(f8)
        m["x16"] = np.ascontiguousarray(
            xsh[c].reshape(BLOC, NJ, 128, D).transpose(0, 2, 1, 3)
            .reshape(BLOC, 128, NJ * D)).astype(np.float16)
        in_maps.append(m)
    return in_maps


def _run(in_maps, trace=False, tmpdir=None):
    from concourse.bass_utils import run_bass_kernel_spmd
    if "nc" not in _CACHE:
        _CACHE["nc"] = _build_nc()
    kw = {}
    if trace:
        _install_ntff_hook()
        kw = dict(trace=True, tmpdir=tmpdir)
    return run_bass_kernel_spmd(_CACHE["nc"], in_maps,
                                core_ids=list(range(NCORES)), **kw)


def _install_ntff_hook():
    import types, importlib.util as ilu
    if "antenv.axon_hooks" in sys.modules:
        return
    spec = ilu.spec_from_file_location(
        "trn_boot_mod", "/root/.axon_site/trn_agent_boot/trn_boot.py")
    tb = ilu.module_from_spec(spec)
    spec.loader.exec_module(tb)
    hook = tb._ntff_profile_via_ctypes("/opt/axon/libaxon_pjrt.so")
    mod = types.ModuleType("antenv.axon_hooks")
    mod.get_axon_ntff_profile_hook = lambda: hook
    import antenv  # noqa: F401
    sys.modules["antenv.axon_hooks"] = mod


def kernel(**inputs) -> np.ndarray:
    in_maps = _prep_inputs(**inputs)
    r = _run(in_maps)
    outs = [np.ascontiguousarray(
        r.results[c]["out"].transpose(0, 2, 1)).astype(np.float32)
        for c in range(NCORES)]
    return np.concatenate(outs, axis=0)
